# revision 1
# baseline (speedup 1.0000x reference)
"""MemoryBank kernel for 8x TRN2 NeuronCores (SPMD, batch-parallel).

Algebraic restructure (exact in real arithmetic):
    scores   = x @ (memory @ key_w).T            # fold key proj into 64-slot table
    gate_x   = x @ gate_w[:, :D].T               # ridden along as a 65th score row
    attn     = softmax(scores / 0.1)
    retrieved= attn @ (memory @ value_w.T)       # fold value proj into table
    ret_gate = attn @ (memory @ value_w.T @ gate_w[:, D:].T)
    gate     = sigmoid(gate_x + ret_gate + b)
    out      = gate * x + (1 - gate) * retrieved

This cuts matmul FLOPs 8x vs the unfused graph and makes the kernel
DMA-bound (read x once, write out once). Each core handles one batch
element, fully dim-major (x pre-transposed on host), with fp32 scores
via float32r full-rate matmuls so the sharp softmax (logit std ~226)
keeps exact fp32 argmax behavior.
"""

from contextlib import ExitStack

import numpy as np

import concourse.bass as bass
import concourse.tile as tile
from concourse import bacc
from concourse import mybir
from concourse.bass import ts
from concourse.bass_utils import run_bass_kernel_spmd
from concourse.masks import make_identity

F32 = mybir.dt.float32
F32R = mybir.dt.float32r
AX_X = mybir.AxisListType
ALU = mybir.AluOpType
ACTF = mybir.ActivationFunctionType

B = 8
L = 4096
DIM = 2048
NSLOT = 64
NCH = DIM // 128  # 16 dim chunks
TOK = 512  # tokens per tile
NT = L // TOK  # 8 tiles per core
NQ = TOK // 128  # 4 token quarters per tile


def _build(gate_b: float, use_f32r: bool = True, s_f32r: bool = False) -> bass.Bass:
    nc = bacc.Bacc("TRN2", target_bir_lowering=False, debug=False)
    FR = F32R if use_f32r else F32
    FS = F32R if s_f32r else F32

    xT = nc.dram_tensor("xT", [DIM, L], FS, kind="ExternalInput").ap()
    GT = nc.dram_tensor("GT", [DIM, NSLOT + 1], FS, kind="ExternalInput").ap()
    WvN = nc.dram_tensor("WvN", [NSLOT, DIM], F32, kind="ExternalInput").ap()
    gv = nc.dram_tensor("gv", [1, NSLOT], F32, kind="ExternalInput").ap()
    outT = nc.dram_tensor("outT", [DIM, L], F32, kind="ExternalOutput").ap()

    # dim d = c*128 + p  (chunk-major split; consistent everywhere)
    xT_v = xT.rearrange("(c p) t -> p c t", p=128)  # [128, 16, L]
    GT_v = GT.rearrange("(c p) m -> p c m", p=128)  # [128, 16, 65]
    outT_v = outT.rearrange("(c p) t -> p c t", p=128)
    WvN_v = WvN.rearrange("n (c q) -> n c q", q=128)  # [64, 16, 128]

    with tile.TileContext(nc) as tc, ExitStack() as ctx:
        consts = ctx.enter_context(tc.tile_pool(name="consts", bufs=1))
        xpool = ctx.enter_context(tc.tile_pool(name="xpool", bufs=3))
        opool = ctx.enter_context(tc.tile_pool(name="opool", bufs=2))
        work = ctx.enter_context(tc.tile_pool(name="work", bufs=3))
        small = ctx.enter_context(tc.tile_pool(name="small", bufs=4))
        psA = ctx.enter_context(tc.tile_pool(name="psA", bufs=2, space="PSUM"))
        psT = ctx.enter_context(tc.tile_pool(name="psT", bufs=2, space="PSUM"))
        psE = ctx.enter_context(tc.tile_pool(name="psE", bufs=1, space="PSUM"))
        psR = ctx.enter_context(tc.tile_pool(name="psR", bufs=2, space="PSUM"))
        psG = ctx.enter_context(tc.tile_pool(name="psG", bufs=1, space="PSUM"))

        ident = consts.tile([128, 128], F32)
        make_identity(nc, ident)
        GT_sb = consts.tile([128, NCH, NSLOT + 1], FS)
        nc.sync.dma_start(out=GT_sb, in_=GT_v)
        Wv_stage = consts.tile([NSLOT, NCH, 128], F32)
        nc.sync.dma_start(out=Wv_stage, in_=WvN_v)
        Wv_sb = consts.tile([NSLOT, NCH, 128], FR)
        nc.scalar.copy(Wv_sb, Wv_stage)
        gv_rep = consts.tile([128, NSLOT], F32)
        nc.sync.dma_start(out=gv_rep, in_=gv.to_broadcast((128, NSLOT)))
        ones_t = consts.tile([NSLOT + 1, 128], FR)
        # memset can't target f32r; ACT Copy with scale=0 bias=1 writes ones
        nc.scalar.activation(
            ones_t, ident[0 : NSLOT + 1, :], func=ACTF.Copy, bias=1.0, scale=0.0
        )

        def phase_A_dma(t):
            """prefetch x^T tile (2 tiles ahead of use)."""
            x_sb = xpool.tile([128, NCH, TOK], FS, tag="x_sb")
            nc.sync.dma_start(out=x_sb, in_=xT_v[:, :, ts(t, TOK)])
            return {"x_sb": x_sb}

        def phase_A_mm(t, st):
            """S^T matmuls -> ACT copy -> token-major transposes."""
            x_sb = st["x_sb"]
            S_ps = psA.tile([128, TOK], F32, tag="A")
            for c in range(NCH):
                nc.tensor.matmul(
                    S_ps[0 : NSLOT + 1, :],
                    GT_sb[:, c, :],
                    x_sb[:, c, :],
                    start=(c == 0),
                    stop=(c == NCH - 1),
                )
            S_sb = work.tile([NSLOT + 1, TOK], F32, tag="S_sb")
            nc.scalar.copy(S_sb, S_ps[0 : NSLOT + 1, :])
            Stok = psT.tile([128, NQ, NSLOT + 1], F32, tag="T")
            for q in range(NQ):
                nc.tensor.transpose(
                    Stok[:, q, :],
                    S_sb[:, ts(q, 128)],
                    ident[0 : NSLOT + 1, 0 : NSLOT + 1],
                )
            st["Stok"] = Stok

        def phase_B(t, st):
            """Per-token-quarter softmax stats (DVE/ACT only)."""
            Stok = st["Stok"]
            Etok = small.tile([128, NQ, NSLOT], F32, tag="Etok")
            Ec = small.tile([128, NQ, NSLOT + 1], F32, tag="Ec")
            # ACT functions are clustered (all Exp, later all Sigmoid): every
            # function switch costs a ~1.3us ACT_TABLE_LOAD on this runtime.
            mbs, rs, gls, gts = [], [], [], []
            for q in range(NQ):
                St = Stok[:, q, :]  # cols 0..63 = scores, col 64 = gate_x
                mx = small.tile([128, 1], F32, tag="mx")
                nc.vector.tensor_reduce(mx, St[:, 0:NSLOT], axis=AX_X.X, op=ALU.max)
                mb = small.tile([128, 1], F32, tag="mb")
                nc.vector.tensor_scalar_mul(mb, mx, -10.0)
                mbs.append(mb)
            for q in range(NQ):
                nc.scalar.activation(
                    Etok[:, q, :],
                    Stok[:, q, 0:NSLOT],
                    func=ACTF.Exp,
                    bias=mbs[q],
                    scale=10.0,
                )
            for q in range(NQ):
                St = Stok[:, q, :]
                sums = small.tile([128, 1], F32, tag="sums")
                nc.vector.tensor_reduce(sums, Etok[:, q, :], axis=AX_X.X, op=ALU.add)
                scr = small.tile([128, NSLOT], F32, tag="scr")
                nc.vector.tensor_mul(scr, Etok[:, q, :], gv_rep)
                gvd = small.tile([128, 1], F32, tag="gvd")
                nc.vector.tensor_reduce(gvd, scr, axis=AX_X.X, op=ALU.add)
                r = small.tile([128, 1], F32, tag="r")
                nc.vector.reciprocal(r, sums)
                t1 = small.tile([128, 1], F32, tag="t1")
                nc.vector.tensor_mul(t1, gvd, r)
                gl = small.tile([128, 1], F32, tag="gl")
                nc.vector.tensor_add(gl, t1, St[:, NSLOT : NSLOT + 1])
                rs.append(r)
                gls.append(gl)
            for q in range(NQ):
                g_t = small.tile([128, 1], F32, tag="g_t")
                nc.scalar.activation(
                    g_t, gls[q], func=ACTF.Sigmoid, bias=gate_b, scale=1.0
                )
                gts.append(g_t)
            for q in range(NQ):
                r, g_t = rs[q], gts[q]
                nc.vector.tensor_copy(Ec[:, q, NSLOT : NSLOT + 1], g_t)
                t2 = small.tile([128, 1], F32, tag="t2")
                nc.vector.tensor_mul(t2, g_t, r)
                cp = small.tile([128, 1], F32, tag="cp")
                nc.vector.tensor_sub(cp, t2, r)
                nc.vector.tensor_scalar_mul(Ec[:, q, 0:NSLOT], Etok[:, q, :], cp)
            st["Ec"] = Ec

        def phase_C(t, st, s_next=None):
            """Back to slot-major, g broadcast, retrieval + combine, out DMA.

            s_next=(t2, st2): interleave tile t2's S-matmul accumulation
            between this tile's R matmuls so PE never idles while DVE
            paces the combine (PSUM groups are per-bank, so this is legal).
            """
            x_sb, Ec = st["x_sb"], st["Ec"]
            ET = psE.tile([NSLOT + 1, NQ, 128], F32, tag="E")
            for q in range(NQ):
                nc.tensor.transpose(ET[:, q, :], Ec[:, q, :], ident)
            E_sb = work.tile([NSLOT + 1, NQ, 128], FR, tag="E_sb")
            nc.scalar.copy(E_sb, ET)
            E_flat = E_sb.rearrange("p a b -> p (a b)")  # [65, 512]
            g_bc = psG.tile([128, TOK], F32, tag="G")
            nc.tensor.matmul(
                g_bc,
                ones_t[NSLOT : NSLOT + 1, :],
                E_flat[NSLOT : NSLOT + 1, :],
                start=True,
                stop=True,
            )
            g_sb = work.tile([128, TOK], F32, tag="g_sb")
            nc.scalar.copy(g_sb, g_bc)
            if s_next is not None:
                t2, st2 = s_next
                S_ps2 = psA.tile([128, TOK], F32, tag="A")
            for a in range(4):
                o4 = opool.tile([128, 4, TOK], F32, tag="o4")
                for cc in range(4):
                    c = 4 * a + cc
                    R_ps = psR.tile([128, TOK], F32, tag="R")
                    nc.tensor.matmul(
                        R_ps,
                        Wv_sb[:, c, :],
                        E_flat[0:NSLOT, :],
                        start=True,
                        stop=True,
                    )
                    if s_next is not None:
                        nc.tensor.matmul(
                            S_ps2[0 : NSLOT + 1, :],
                            GT_sb[:, c, :],
                            st2["x_sb"][:, c, :],
                            start=(c == 0),
                            stop=(c == NCH - 1),
                            skip_group_check=True,
                        )
                    xg = work.tile([128, TOK], F32, tag="xg")
                    xin = x_sb[:, c, :].bitcast(F32) if s_f32r else x_sb[:, c, :]
                    if c < 10:
                        nc.gpsimd.tensor_mul(xg, xin, g_sb)
                    else:
                        nc.vector.tensor_mul(xg, xin, g_sb)
                    nc.vector.tensor_add(o4[:, cc, :], xg, R_ps)
                nc.sync.dma_start(
                    out=outT_v[:, 4 * a : 4 * a + 4, ts(t, TOK)],
                    in_=o4,
                )
            if s_next is not None:
                S_sb2 = work.tile([NSLOT + 1, TOK], F32, tag="S_sb")
                nc.scalar.copy(S_sb2, S_ps2[0 : NSLOT + 1, :])
                Stok2 = psT.tile([128, NQ, NSLOT + 1], F32, tag="T")
                for q in range(NQ):
                    nc.tensor.transpose(
                        Stok2[:, q, :],
                        S_sb2[:, ts(q, 128)],
                        ident[0 : NSLOT + 1, 0 : NSLOT + 1],
                    )
                st2["Stok"] = Stok2

        # software pipeline: x-DMA 2 tiles ahead; tile t+2's S matmuls are
        # interleaved into tile t's R phase (phase_C) so PE never idles.
        states = {}
        states[0] = phase_A_dma(0)
        states[1] = phase_A_dma(1)
        phase_A_mm(0, states[0])
        phase_A_mm(1, states[1])
        phase_B(0, states[0])
        for t in range(NT):
            if t + 2 < NT:
                states[t + 2] = phase_A_dma(t + 2)
            if t + 1 < NT:
                # stats for t+1 go on DVE *before* tile t's combine so the
                # PE never stalls on Ec at the next tile boundary
                phase_B(t + 1, states[t + 1])
            phase_C(
                t,
                states[t],
                s_next=(t + 2, states[t + 2]) if t + 2 < NT else None,
            )
            del states[t]

    nc.compile()
    return nc


def _fold_weights(memory, key_w, value_w, gate_w):
    mem = memory.astype(np.float64)
    Ws = (mem @ key_w.astype(np.float64)).astype(np.float32)  # [64, 2048]
    Wv = (mem @ value_w.astype(np.float64).T).astype(np.float32)  # [64, 2048]
    gx = np.asarray(gate_w[0, :DIM], dtype=np.float32)
    gvv = (Wv.astype(np.float64) @ gate_w[0, DIM:].astype(np.float64)).astype(
        np.float32
    )
    G = np.concatenate([Ws, gx[None, :]], axis=0)  # [65, 2048]; gate row last
    GT = np.ascontiguousarray(G.T)  # [2048, 65]
    WvN = np.ascontiguousarray(-Wv)  # [64, 2048]
    return GT, WvN, gvv.reshape(1, NSLOT)


def kernel(x, memory, key_w, value_w, gate_w, gate_b, _trace=False, _tmpdir=None, _use_f32r=True, _s_f32r=False):
    x = np.asarray(x, dtype=np.float32)
    GT, WvN, gvv = _fold_weights(
        np.asarray(memory, np.float32),
        np.asarray(key_w, np.float32),
        np.asarray(value_w, np.float32),
        np.asarray(gate_w, np.float32),
    )
    nc = _build(float(np.asarray(gate_b).reshape(-1)[0]), use_f32r=_use_f32r, s_f32r=_s_f32r)
    in_maps = [
        {"xT": np.ascontiguousarray(x[b].T), "GT": GT, "WvN": WvN, "gv": gvv}
        for b in range(B)
    ]
    res = run_bass_kernel_spmd(
        nc, in_maps, list(range(B)), trace=_trace, tmpdir=_tmpdir
    )
    out = np.stack([res.results[b]["outT"].T for b in range(B)], axis=0)
    if _trace:
        return out.astype(np.float32), res
    return out.astype(np.float32)

